# revision 4
# baseline (speedup 1.0000x reference)
"""Trainium2 Bass kernel for DyIntraModalityUpdate (dense_transformer).

Strategy: pure data-parallel over batch (32 batches -> 4 per core x 8 cores).
Both modalities (v, q) of a batch live on the same core, so the cross-modal
sigmoid gates need no communication at all.

Per-core kernel (all matmuls bf16 with f32 PSUM accumulation):
  - activations flow in transposed layout [channels, tokens] so that:
      * per-head K^T / Q^T tiles are natural [Dh=128, tok] slices
      * softmax is a free-dim reduction
      * gates + biases fuse into per-partition ACT epilogues
        (relu(g*x + g*b) == g*relu(x+b), g = 1+sigmoid > 0)
  - V is produced in natural [tok, ch] layout (bias added via a K=1
    ones-row matmul) so P @ V needs only a PE transpose of P.
  - the value-gate commutes out of P@V (per-channel), applied in the
    fused residual epilogue: resid^T = g * (P@V)^T + x^T (one DVE op).
  - final projection computed transposed (per-partition bias), host
    un-transposes the f32 result.

Host side: shards batch, pre-casts to bf16, pre-transposes inputs,
runs SPMD on 8 cores via run_bass_kernel_spmd, gathers + un-transposes.
"""

import sys
from contextlib import ExitStack

import numpy as np

for _p in ("/opt/trn_rl_repo",):
    if _p not in sys.path:
        sys.path.insert(0, _p)

# Full-size problem config (hardcoded per harness contract).
N_CORES = 8
B_GLOBAL = 32
NV = 512  # num_obj (v tokens)
NQ = 256  # max_len (q tokens)
C = 1024  # v_size == q_size
OUT = 1024
H = 8  # heads
DH = OUT // H  # 128


def build_program(B, Nv, Nq, C, OUT, H, n_cores=8):
    import concourse.bass as bass
    import concourse.tile as tile
    from concourse import bacc, mybir
    from concourse.masks import make_identity

    dt = mybir.dt
    bf16 = dt.bfloat16
    f32 = dt.float32
    AF = mybir.ActivationFunctionType
    ALU = mybir.AluOpType
    AX = mybir.AxisListType

    assert OUT // H == 128, "head dim must be 128"
    KT = C // 128  # contraction tiles over input channels
    HT = OUT // 128  # tiles over OUT (== heads)
    O3 = 3 * OUT
    assert C == OUT, "residual layout assumes C == OUT"

    nc = bacc.Bacc(
        "TRN2", target_bir_lowering=False, debug=False, num_devices=n_cores
    )

    # ---- DRAM I/O ----
    vt_d = nc.dram_tensor("vt", [C, B, Nv], bf16, kind="ExternalInput").ap()
    qt_d = nc.dram_tensor("qt", [C, B, Nq], bf16, kind="ExternalInput").ap()
    wvlin_d = nc.dram_tensor("wvlin", [C, O3], bf16, kind="ExternalInput").ap()
    wqlin_d = nc.dram_tensor("wqlin", [C, O3], bf16, kind="ExternalInput").ap()
    wvout_d = nc.dram_tensor("wvout", [OUT, OUT], bf16, kind="ExternalInput").ap()
    wqout_d = nc.dram_tensor("wqout", [OUT, OUT], bf16, kind="ExternalInput").ap()
    wv4q_d = nc.dram_tensor("wv4q", [C, OUT], bf16, kind="ExternalInput").ap()
    wq4v_d = nc.dram_tensor("wq4v", [C, OUT], bf16, kind="ExternalInput").ap()
    bvlin_c_d = nc.dram_tensor("bvlin_col", [128, 3 * HT], f32, kind="ExternalInput").ap()
    bqlin_c_d = nc.dram_tensor("bqlin_col", [128, 3 * HT], f32, kind="ExternalInput").ap()
    bvval_d = nc.dram_tensor("bvval_row", [1, OUT], bf16, kind="ExternalInput").ap()
    bqval_d = nc.dram_tensor("bqval_row", [1, OUT], bf16, kind="ExternalInput").ap()
    bvout_d = nc.dram_tensor("bvout_col", [128, HT], f32, kind="ExternalInput").ap()
    bqout_d = nc.dram_tensor("bqout_col", [128, HT], f32, kind="ExternalInput").ap()
    bv4q_d = nc.dram_tensor("bv4q_col", [128, HT], f32, kind="ExternalInput").ap()
    bq4v_d = nc.dram_tensor("bq4v_col", [128, HT], f32, kind="ExternalInput").ap()
    ovt_d = nc.dram_tensor("ovt", [B, OUT, Nv], f32, kind="ExternalOutput").ap()
    oqt_d = nc.dram_tensor("oqt", [B, OUT, Nq], f32, kind="ExternalOutput").ap()

    with tile.TileContext(nc) as tc, ExitStack() as ctx:
        consts = ctx.enter_context(tc.tile_pool(name="consts", bufs=1))
        inp = ctx.enter_context(tc.tile_pool(name="inp", bufs=1))
        wbig = ctx.enter_context(tc.tile_pool(name="wbig", bufs=1))
        wsq = ctx.enter_context(tc.tile_pool(name="wsq", bufs=2))
        kqv = ctx.enter_context(tc.tile_pool(name="kqv", bufs=1))
        attn = ctx.enter_context(tc.tile_pool(name="attn", bufs=2))
        stat = ctx.enter_context(tc.tile_pool(name="stat", bufs=8))
        outp = ctx.enter_context(tc.tile_pool(name="outp", bufs=3))
        psum_mm = ctx.enter_context(
            tc.tile_pool(name="psum_mm", bufs=4, space=bass.MemorySpace.PSUM)
        )
        psum_tp = ctx.enter_context(
            tc.tile_pool(name="psum_tp", bufs=2, space=bass.MemorySpace.PSUM)
        )
        psum_sm = ctx.enter_context(
            tc.tile_pool(name="psum_sm", bufs=2, space=bass.MemorySpace.PSUM)
        )

        # ---- constants ----
        ident = consts.tile([128, 128], bf16)
        make_identity(nc, ident[:])
        ones_row = consts.tile([1, 128], bf16)
        nc.gpsimd.memset(ones_row[:], 1.0)

        def load_const(dram_ap, shape, dtype):
            t = consts.tile(shape, dtype, tag=f"c{dram_ap.tensor.name}")
            nc.sync.dma_start(t[:], dram_ap)
            return t

        bvlin_c = load_const(bvlin_c_d, [128, 3 * HT], f32)
        bqlin_c = load_const(bqlin_c_d, [128, 3 * HT], f32)
        bvval = load_const(bvval_d, [1, OUT], bf16)
        bqval = load_const(bqval_d, [1, OUT], bf16)
        bvout_c = load_const(bvout_d, [128, HT], f32)
        bqout_c = load_const(bqout_d, [128, HT], f32)
        bv4q_c = load_const(bv4q_d, [128, HT], f32)
        bq4v_c = load_const(bq4v_d, [128, HT], f32)

        # ---- inputs (transposed activations) ----
        vt_sb = inp.tile([128, KT, B, Nv], bf16, tag="vt")
        nc.sync.dma_start(vt_sb[:], vt_d.rearrange("(k p) b n -> p k b n", p=128))
        qt_sb = inp.tile([128, KT, B, Nq], bf16, tag="qt")
        nc.sync.dma_start(qt_sb[:], qt_d.rearrange("(k p) b n -> p k b n", p=128))

        # ---- means (transposed, bf16 for gate matmul rhs) ----
        def means(xt_sb, Ntok):
            m = consts.tile([128, KT, B], bf16, tag=f"mean{Ntok}")
            for ct in range(KT):
                tmp = stat.tile([128, B], f32, tag="meantmp")
                nc.vector.reduce_sum(tmp[:], xt_sb[:, ct], axis=AX.X)
                nc.vector.tensor_copy(m[:, ct], tmp[:])
            return m

        vmean = means(vt_sb, Nv)  # actually the *sum*; 1/N folded into sigmoid
        qmean = means(qt_sb, Nq)

        # ---- gate weights + gate matmuls ----
        wv4q_sb = wsq.tile([128, KT, OUT], bf16, tag="wsq")
        nc.sync.dma_start(wv4q_sb[:], wv4q_d.rearrange("(k p) n -> p k n", p=128))
        wq4v_sb = wsq.tile([128, KT, OUT], bf16, tag="wsq")
        nc.sync.dma_start(wq4v_sb[:], wq4v_d.rearrange("(k p) n -> p k n", p=128))

        # gate_v gates the v path (from q_mean @ w_q4v); gate_q vice versa
        def gate(w_sb, mean_sb, bcol, inv_n):
            g = consts.tile([128, HT, B], f32, tag=f"g{w_sb.name}")
            for ct in range(HT):
                pg = psum_sm.tile([128, B], f32, tag="pg")
                for kt in range(KT):
                    nc.tensor.matmul(
                        pg[:],
                        w_sb[:, kt, ct * 128 : (ct + 1) * 128],
                        mean_sb[:, kt],
                        start=kt == 0,
                        stop=kt == KT - 1,
                    )
                nc.scalar.activation(
                    g[:, ct], pg[:], AF.Sigmoid, bias=bcol[:, ct : ct + 1], scale=inv_n
                )
            return g

        gate_v = gate(wq4v_sb, qmean, bq4v_c, 1.0 / Nq)
        gate_q = gate(wv4q_sb, vmean, bv4q_c, 1.0 / Nv)

        # ---- derived per-channel scales/biases ----
        def derive(gate_t, blin_c, tag):
            g1 = consts.tile([128, HT, B], f32, tag=f"g1{tag}")
            nc.vector.tensor_scalar_add(g1[:], gate_t[:], 1.0)
            gq = consts.tile([128, HT, B], f32, tag=f"gq{tag}")
            nc.vector.tensor_scalar_mul(gq[:], g1[:], float(DH) ** -0.5)
            bK = consts.tile([128, HT, B], f32, tag=f"bK{tag}")
            bQ = consts.tile([128, HT, B], f32, tag=f"bQ{tag}")
            for ct in range(HT):
                nc.vector.tensor_scalar(
                    bK[:, ct], g1[:, ct], blin_c[:, ct : ct + 1], None, op0=ALU.mult
                )
                nc.vector.tensor_scalar(
                    bQ[:, ct], gq[:, ct], blin_c[:, HT + ct : HT + ct + 1], None,
                    op0=ALU.mult,
                )
            return g1, gq, bK, bQ

        g1_v, gq_v, bK_v, bQ_v = derive(gate_v, bvlin_c, "v")
        g1_q, gq_q, bK_q, bQ_q = derive(gate_q, bqlin_c, "q")

        # ---- big weights ----
        def load_w3(dram_ap):
            t = wbig.tile([128, KT, O3], bf16, tag="w3")
            nc.sync.dma_start(t[:], dram_ap.rearrange("(k p) n -> p k n", p=128))
            return t

        def load_wout(dram_ap):
            t = wsq.tile([128, HT, OUT], bf16, tag="wsq")
            nc.sync.dma_start(t[:], dram_ap.rearrange("(k p) n -> p k n", p=128))
            return t

        def modality(tag, Ntok, xt, wlin_sb, wout_sb, bval, g1, gq, bK, bQ, bout_c, out_d):
            NT = Ntok // 128
            for b in range(B):
                # --- QKV projections ---
                k_sb = kqv.tile([128, HT, Ntok], bf16, tag="k")
                qp_sb = kqv.tile([128, HT, Ntok], bf16, tag="qp")
                vval = kqv.tile([128, NT, OUT], bf16, tag="vv")
                for sec, dest, gs, gb in ((0, k_sb, g1, bK), (1, qp_sb, gq, bQ)):
                    for ct in range(HT):
                        ps = psum_mm.tile([128, Ntok], f32, tag="ps")
                        mo = sec * OUT + ct * 128
                        for kt in range(KT):
                            nc.tensor.matmul(
                                ps[:],
                                wlin_sb[:, kt, mo : mo + 128],
                                xt[:, kt, b],
                                start=kt == 0,
                                stop=kt == KT - 1,
                            )
                        nc.scalar.activation(
                            dest[:, ct], ps[:], AF.Relu,
                            bias=gb[:, ct, b : b + 1], scale=gs[:, ct, b : b + 1],
                        )
                # value in natural [tok, ch] layout; bias via ones-row matmul
                NW = min(512, OUT)
                for tt in range(NT):
                    for nt in range(OUT // NW):
                        ps = psum_mm.tile([128, NW], f32, tag="ps")
                        no = 2 * OUT + nt * NW
                        for kt in range(KT):
                            nc.tensor.matmul(
                                ps[:],
                                xt[:, kt, b, tt * 128 : (tt + 1) * 128],
                                wlin_sb[:, kt, no : no + NW],
                                start=kt == 0,
                                stop=False,
                            )
                        nc.tensor.matmul(
                            ps[:],
                            ones_row[:],
                            bval[:, nt * NW : (nt + 1) * NW],
                            start=False,
                            stop=True,
                        )
                        nc.scalar.activation(
                            vval[:, tt, nt * NW : (nt + 1) * NW], ps[:], AF.Relu
                        )

                # --- attention per head ---
                resid = kqv.tile([128, HT, Ntok], bf16, tag="r")
                for h in range(H):
                    p_sb = attn.tile([128, NT, Ntok], bf16, tag="p")
                    for qt in range(NT):
                        pl = psum_mm.tile([128, Ntok], f32, tag="ps")
                        nc.tensor.matmul(
                            pl[:],
                            qp_sb[:, h, qt * 128 : (qt + 1) * 128],
                            k_sb[:, h],
                            start=True,
                            stop=True,
                        )
                        nmax = stat.tile([128, 1], f32, tag="nmax")
                        nc.vector.reduce_max(nmax[:], pl[:], axis=AX.X, negate=True)
                        ssum = stat.tile([128, 1], f32, tag="ssum")
                        nc.scalar.activation(
                            p_sb[:, qt], pl[:], AF.Exp,
                            bias=nmax[:], scale=1.0, accum_out=ssum[:],
                        )
                        rsum = stat.tile([128, 1], f32, tag="rsum")
                        nc.vector.reciprocal(rsum[:], ssum[:])
                        nc.vector.tensor_scalar_mul(p_sb[:, qt], p_sb[:, qt], rsum[:])
                    # P^T via PE transpose
                    pT = attn.tile([128, NT, Ntok], bf16, tag="pt")
                    for kt in range(NT):
                        ptp = psum_tp.tile([128, Ntok], bf16, tag="ptp")
                        for qt in range(NT):
                            nc.tensor.transpose(
                                ptp[:, qt * 128 : (qt + 1) * 128],
                                p_sb[:, qt, kt * 128 : (kt + 1) * 128],
                                ident[:],
                            )
                        nc.vector.tensor_copy(pT[:, kt], ptp[:])
                    # P @ V (transposed out), fused gate + residual
                    po = psum_mm.tile([128, Ntok], f32, tag="ps")
                    for kt in range(NT):
                        nc.tensor.matmul(
                            po[:],
                            vval[:, kt, h * 128 : (h + 1) * 128],
                            pT[:, kt],
                            start=kt == 0,
                            stop=kt == NT - 1,
                        )
                    nc.vector.scalar_tensor_tensor(
                        resid[:, h],
                        in0=po[:],
                        scalar=g1[:, h, b : b + 1],
                        in1=xt[:, h, b],
                        op0=ALU.mult,
                        op1=ALU.add,
                    )

                # --- output projection (transposed out) ---
                for ct in range(HT):
                    pf = psum_mm.tile([128, Ntok], f32, tag="ps")
                    for kt in range(HT):
                        nc.tensor.matmul(
                            pf[:],
                            wout_sb[:, kt, ct * 128 : (ct + 1) * 128],
                            resid[:, kt],
                            start=kt == 0,
                            stop=kt == HT - 1,
                        )
                    ot = outp.tile([128, Ntok], f32, tag="o")
                    nc.scalar.activation(
                        ot[:], pf[:], AF.Relu, bias=bout_c[:, ct : ct + 1], scale=1.0
                    )
                    nc.sync.dma_start(out_d[b, ct * 128 : (ct + 1) * 128], ot[:])

        wvlin_sb = load_w3(wvlin_d)
        wvout_sb = load_wout(wvout_d)
        modality("v", Nv, vt_sb, wvlin_sb, wvout_sb, bvval, g1_v, gq_v, bK_v, bQ_v,
                 bvout_c, ovt_d)
        wqlin_sb = load_w3(wqlin_d)
        wqout_sb = load_wout(wqout_d)
        modality("q", Nq, qt_sb, wqlin_sb, wqout_sb, bqval, g1_q, gq_q, bK_q, bQ_q,
                 bqout_c, oqt_d)

    nc.compile()
    return nc


def make_in_map(v_shard, q_shard, w, bf):
    """Build the per-core input map from f32 numpy shards + weight dict."""

    def colsplit(b):  # [n*128] -> [128, n] with col j = b[j*128:(j+1)*128]
        return np.ascontiguousarray(b.reshape(-1, 128).T).astype(np.float32)

    return dict(
        vt=np.ascontiguousarray(v_shard.transpose(2, 0, 1)).astype(bf),
        qt=np.ascontiguousarray(q_shard.transpose(2, 0, 1)).astype(bf),
        wvlin=w["w_vlin"].astype(bf),
        wqlin=w["w_qlin"].astype(bf),
        wvout=w["w_vout"].astype(bf),
        wqout=w["w_qout"].astype(bf),
        wv4q=w["w_v4q"].astype(bf),
        wq4v=w["w_q4v"].astype(bf),
        bvlin_col=colsplit(w["b_vlin"]),
        bqlin_col=colsplit(w["b_qlin"]),
        bvval_row=np.ascontiguousarray(w["b_vlin"][None, -w["w_vout"].shape[0]:]).astype(bf),
        bqval_row=np.ascontiguousarray(w["b_qlin"][None, -w["w_qout"].shape[0]:]).astype(bf),
        bvout_col=colsplit(w["b_vout"]),
        bqout_col=colsplit(w["b_qout"]),
        bv4q_col=colsplit(w["b_v4q"]),
        bq4v_col=colsplit(w["b_q4v"]),
    )


_CACHE = {}


def _compiled():
    if "nc" not in _CACHE:
        _CACHE["nc"] = build_program(
            B=B_GLOBAL // N_CORES, Nv=NV, Nq=NQ, C=C, OUT=OUT, H=H, n_cores=N_CORES
        )
    return _CACHE["nc"]


def kernel(**inputs):
    import ml_dtypes

    from concourse.bass_utils import run_bass_kernel_spmd

    bf = ml_dtypes.bfloat16
    nc = _compiled()
    v = np.asarray(inputs["v"], dtype=np.float32)
    q = np.asarray(inputs["q"], dtype=np.float32)
    Bl = B_GLOBAL // N_CORES
    in_maps = [
        make_in_map(v[i * Bl : (i + 1) * Bl], q[i * Bl : (i + 1) * Bl], inputs, bf)
        for i in range(N_CORES)
    ]
    res = run_bass_kernel_spmd(nc, in_maps, list(range(N_CORES)))
    if res.exec_time_ns is not None:
        print(f"HW exec time: {res.exec_time_ns} ns")
    upd_v = np.concatenate(
        [res.results[i]["ovt"].transpose(0, 2, 1) for i in range(N_CORES)], axis=0
    ).astype(np.float32)
    upd_q = np.concatenate(
        [res.results[i]["oqt"].transpose(0, 2, 1) for i in range(N_CORES)], axis=0
    ).astype(np.float32)
    return upd_v, upd_q
